# revision 27
# baseline (speedup 1.0000x reference)
"""EntityCrossAttention Trainium2 kernel (bf16 streaming, transposed output).

Reference computation (per batch b):
    E = noun_feats[class_ids[b]]            [N, D]
    Q = X @ Wq.T + bq                       [T, D]
    K = E @ Wk.T + bk                       [N, D]
    V = E @ Wv.T + bv                       [N, D]
    S = Q @ K.T / sqrt(D)                   [T, N]
    attn = softmax(S, -1)
    wa = attn * w;  wa /= wa.sum(-1) + 1e-6
    out = wa @ V                            [T, D]

Algebraic restructuring: S = X @ (Wq.T @ K.T)/sqrt(D) + (bq @ K.T)/sqrt(D), so
the [D,D] Q projection never exists on device. Host precomputes per batch:
    M  = Wq.T @ K.T               [D, N]
    eb = (bq @ K.T) / sqrt(D)     [N]
    V' = w[:,None] * V            [N, D]
With unnormalized weights e = exp(S/sqrt(D) + eb):
    out = (e @ V') / (e @ (w + 1e-6))

Device computes the two big contractions only; the tiny per-row denominator
e @ (w+1e-6) is evaluated on the host from the shipped e (bf16, 0.5 MiB/core),
and the final division happens on the host. All PE operands are bf16
(1 col/cycle stream), PSUM accumulation f32, X and outputs stream bf16
(~6e-3 max-rel error vs the f32 reference; tolerance is 2e-2).

Per 1024-row group (8 per core):
    scoresT[n, r] : 2 halves x 4 k-chunk matmuls, M chunks stationary [128,32]
    eT = exp(scoresT*scale + eb) -> bf16 SBUF   (ScalarE, 2 instr)
    raw.T[d, r]   : 4 V' chunks stationary [32,128], eT moving [32,512]
                    -> PSUM [128,512] f32 (6-bank rotation)
    casts PSUM->SBUF bf16 split 5/3 across DVE / ScalarE
The out matmuls for group g issue after the score matmuls of group g+1, so the
PE always has a dense run of 512-column bf16 matmuls (keeps the PE activity
throttle at full rate) and the casts decouple through the 6 PSUM banks.

Sharding: data-parallel over B: 8 cores x 2 batches each. X loads on the SP
HWDGE ring (512 KiB halves, 4 KiB/partition contiguous), raw.T and e stores on
the GpSimd SWDGE ring. Host reassembles and applies the denominator.
"""

import numpy as np

B, T, D, C, N = 16, 4096, 512, 14, 32
N_CORES = 8
B_PC = B // N_CORES          # batches per core
ROWS_PC = B_PC * T           # 8192
GR = 1024                    # rows per group (one 1 MiB DMA each way in bf16)
NG = ROWS_PC // GR           # 8 groups per core
GPB = T // GR                # 4 groups per batch
SH = 512                     # scores half width (PSUM bank / matmul N limit)
KC = D // 128                # 4 contraction chunks
DC = D // 128                # 4 output d-chunks
SCALE = float(D) ** -0.5

_compiled = None


def _build():
    import concourse.bacc as bacc
    import concourse.tile as tile
    import concourse.mybir as mybir

    f32 = mybir.dt.float32
    bf16 = mybir.dt.bfloat16
    Exp = mybir.ActivationFunctionType.Exp
    Copy = mybir.ActivationFunctionType.Copy

    nc = bacc.Bacc("TRN2", debug=False)
    x = nc.dram_tensor("x", [128, NG * 2 * KC * SH], bf16, kind="ExternalInput").ap()
    m = nc.dram_tensor("m", [128, B_PC * KC * N], bf16, kind="ExternalInput").ap()
    vp = nc.dram_tensor("vp", [N, B_PC * D], bf16, kind="ExternalInput").ap()
    eb = nc.dram_tensor("eb", [N, B_PC], f32, kind="ExternalInput").ap()
    out = nc.dram_tensor("out", [128, NG * 2 * DC * SH], bf16,
                         kind="ExternalOutput").ap()
    eo = nc.dram_tensor("eo", [N, NG * GR], bf16, kind="ExternalOutput").ap()

    HCOL = KC * SH  # x columns per half-group load

    with tile.TileContext(nc) as tc:
        with (
            tc.tile_pool(name="const", bufs=1) as cpool,
            tc.tile_pool(name="xin", bufs=2 * NG - 4) as xpool,
            tc.tile_pool(name="xin0", bufs=16) as x0pool,
            tc.tile_pool(name="et", bufs=NG) as epool,
            tc.tile_pool(name="res", bufs=6) as rpool,
            tc.tile_pool(name="ps_sc", bufs=1, space="PSUM") as ps_sc,
            tc.tile_pool(name="ps_o", bufs=6, space="PSUM") as ps_o,
        ):
            # tiny constants first so the first score matmul is not gated on
            # the full X stream; then queue every X group load on the SP ring
            m_sb = cpool.tile([128, B_PC * KC * N], bf16)
            nc.sync.dma_start(m_sb[:, :], m[:, :])
            vp_sb = cpool.tile([N, B_PC * D], bf16)
            nc.sync.dma_start(vp_sb[:, :], vp[:, :])
            eb_sb = cpool.tile([N, B_PC], f32)
            nc.sync.dma_start(eb_sb[:, :], eb[:, :])
            # ramp: the ACT ring is idle until the first store (~20us), so
            # the first two groups' h1 halves load there in parallel with the
            # SP ring. Those first halves also load at k-chunk granularity
            # (4 x 128 KiB into separate tiles) so the first score matmul
            # starts as soon as 128 KiB has landed.
            x_sb = []
            for gi in range(NG):
                pair = []
                for hh in range(2):
                    j = gi * 2 + hh
                    eng = nc.scalar if (gi < 2 and hh == 1) else nc.sync
                    if gi < 2:
                        ks = []
                        for k in range(KC):
                            xk = x0pool.tile([128, SH], bf16,
                                             name="xk_sb", tag="xk_sb")
                            eng.dma_start(
                                xk[:, :],
                                x[:, j * HCOL + k * SH : j * HCOL + (k + 1) * SH],
                            )
                            ks.append(xk)
                        pair.append(ks)
                    else:
                        xt = xpool.tile([128, HCOL], bf16,
                                        name="x_sb", tag="x_sb")
                        eng.dma_start(xt[:, :], x[:, j * HCOL : (j + 1) * HCOL])
                        pair.append(xt)
                x_sb.append(pair)

            def x_rhs(gi, h, k):
                xt = x_sb[gi][h]
                if isinstance(xt, list):
                    return xt[k][:, :]
                return xt[:, k * SH : (k + 1) * SH]

            e_sb = [None] * NG

            # cast split: 5 on DVE, 3 on ScalarE per group of 8
            CAST_ENG = ["v", "v", "s", "v", "s", "v", "s", "v"]

            def scores_stage(gi):
                b = gi // GPB
                e_sb[gi] = epool.tile([N, GR], bf16, name="e_sb", tag="e_sb")
                sc_ps = ps_sc.tile([N, 2 * SH], f32, name="sc_ps", tag="sc_ps")
                for h in range(GR // SH):
                    for k in range(KC):
                        nc.tensor.matmul(
                            sc_ps[:, h * SH : (h + 1) * SH],
                            m_sb[:, (b * KC + k) * N : (b * KC + k + 1) * N],
                            x_rhs(gi, h, k),
                            start=(k == 0),
                            stop=(k == KC - 1),
                        )
                # one batched exp over both halves (FD=1024 on ScalarE)
                nc.scalar.activation(
                    e_sb[gi][:, :], sc_ps[:, :], Exp,
                    bias=eb_sb[:, b : b + 1], scale=SCALE,
                )
                # ship e on the SWDGE ring (pure host output)
                nc.gpsimd.dma_start(
                    eo[:, gi * GR : (gi + 1) * GR], e_sb[gi][:, :]
                )

            def out_stage(gi):
                b = gi // GPB
                o_sb = rpool.tile([128, 2 * DC * SH], bf16,
                                  name="o_sb", tag="o_sb")
                last = gi == NG - 1
                for i in range(8):
                    c, ho = divmod(i, 2)
                    o_ps = ps_o.tile([128, SH], f32, name="o_ps", tag="o_ps")
                    nc.tensor.matmul(
                        o_ps[:, :],
                        vp_sb[:, b * D + c * 128 : b * D + (c + 1) * 128],
                        e_sb[gi][:, ho * SH : (ho + 1) * SH],
                        start=True, stop=True,
                    )
                    dst = o_sb[:, i * SH : (i + 1) * SH]
                    # final group: strict alternation drains the tail casts on
                    # both engines in parallel
                    eng_i = ("v" if i % 2 == 0 else "s") if last else CAST_ENG[i]
                    if eng_i == "v":
                        nc.vector.tensor_copy(dst, o_ps[:, :])
                    else:
                        nc.scalar.activation(dst, o_ps[:, :], Copy)
                    # store finished chunks on the SWDGE ring (quarters for
                    # the last group to shorten the drain tail)
                    ci = i + 1
                    step = 1 if gi == NG - 1 else 4
                    if ci % step == 0:
                        hw = ci // step - 1
                        # alternate stores across the SWDGE and SP rings so
                        # the output stream drains at combined rate; SP-ring
                        # stores queue behind the loads and flow once those
                        # finish, keeping ScalarE free for exp+casts
                        deng = nc.gpsimd if hw % 2 == 0 else nc.sync
                        deng.dma_start(
                            out[:, gi * 2 * DC * SH + hw * step * SH
                                : gi * 2 * DC * SH + (hw + 1) * step * SH],
                            o_sb[:, hw * step * SH : (hw + 1) * step * SH],
                        )

            # software pipeline: out matmuls run one group behind scores
            scores_stage(0)
            for gi in range(1, NG):
                scores_stage(gi)
                out_stage(gi - 1)
            out_stage(NG - 1)

    nc.compile()
    return nc


def _get_compiled():
    global _compiled
    if _compiled is None:
        _compiled = _build()
    return _compiled


def kernel(
    visual_feat, noun_feats, class_ids, noun_weights,
    Wq, bq, Wk, bk, Wv, bv,
):
    import ml_dtypes
    from concourse.bass_utils import run_bass_kernel_spmd

    bf = ml_dtypes.bfloat16
    visual_feat = np.asarray(visual_feat, dtype=np.float32)
    noun_feats = np.asarray(noun_feats, dtype=np.float32)
    class_ids = np.asarray(class_ids)
    noun_weights = np.asarray(noun_weights, dtype=np.float32)
    Wq, bq = np.asarray(Wq, np.float32), np.asarray(bq, np.float32)
    Wk, bk = np.asarray(Wk, np.float32), np.asarray(bk, np.float32)
    Wv, bv = np.asarray(Wv, np.float32), np.asarray(bv, np.float32)

    # Host precompute of tiny per-batch constants (all O(B*N*D)).
    E = noun_feats[class_ids]                       # [B, N, D]
    W = noun_weights[class_ids]                     # [B, N]
    Kb = E @ Wk.T + bk                              # [B, N, D]
    Vb = E @ Wv.T + bv                              # [B, N, D]
    M = np.einsum("jd,bnj->bdn", Wq, Kb)            # [B, D, N] = Wq.T @ Kb.T
    ebias = (Kb @ bq) * SCALE                       # [B, N]
    Vp = W[:, :, None] * Vb                         # [B, N, D]
    wpe = W + 1e-6                                  # [B, N]

    nc = _get_compiled()

    in_maps = []
    for c in range(N_CORES):
        s = slice(c * B_PC, (c + 1) * B_PC)
        # m layout: [128, b*KC*N + k*N + n] = M[b, k*128 + p, n]
        m_c = np.ascontiguousarray(
            M[s].reshape(B_PC, KC, 128, N).transpose(2, 0, 1, 3).reshape(128, -1)
        ).astype(bf)
        # x layout: [p, ((gi*2+h)*KC + k)*SH + r] = Xt[k*128+p, gi*GR+h*SH+r]
        xt_c = visual_feat[s].reshape(ROWS_PC, D).T  # [D, ROWS_PC]
        x_c = np.ascontiguousarray(
            xt_c.reshape(KC, 128, NG, 2, SH)
            .transpose(1, 2, 3, 0, 4).reshape(128, -1)
        ).astype(bf)
        in_maps.append(
            {
                "x": x_c,
                "m": m_c,
                "vp": np.ascontiguousarray(
                    Vp[s].transpose(1, 0, 2).reshape(N, B_PC * D)
                ).astype(bf),
                "eb": np.ascontiguousarray(ebias[s].T),
            }
        )

    global _last_in_maps
    _last_in_maps = in_maps
    res = run_bass_kernel_spmd(nc, in_maps, list(range(N_CORES)))
    # defensive: e is exp(.) so every shipped entry must be positive and
    # finite; retry once if a transient device glitch corrupted an output
    def _bad(r):
        for c in range(N_CORES):
            e_c = np.asarray(r.results[c]["eo"]).astype(np.float32)
            o_c = np.asarray(r.results[c]["out"]).astype(np.float32)
            if not (np.isfinite(e_c).all() and (e_c > 0).all()
                    and np.isfinite(o_c).all() and (o_c != 0.0).all()):
                return True
        return False

    if _bad(res):
        res = run_bass_kernel_spmd(nc, in_maps, list(range(N_CORES)))
    out = np.empty((B, T, D), dtype=np.float32)
    for c in range(N_CORES):
        s = slice(c * B_PC, (c + 1) * B_PC)
        # raw.T dram: [p, ((gi*DC + c)*2 + h)*SH + r] = rawT[d=c*128+p,
        # row=gi*GR+h*SH+r]
        o = np.asarray(res.results[c]["out"]).reshape(128, NG, DC, 2, SH)
        raw = (
            o.transpose(1, 3, 4, 2, 0).reshape(ROWS_PC, D).astype(np.float32)
        )
        # e dram: [n, gi*GR + r] -> den[row] = sum_n e[n,row] * wpe[b(row),n]
        e_c = np.asarray(res.results[c]["eo"]).astype(np.float32)  # [N, 8192]
        wpe_c = wpe[s]                                   # [B_PC, N]
        den = np.einsum(
            "nbr,bn->br", e_c.reshape(N, B_PC, T), wpe_c
        ).reshape(ROWS_PC, 1)
        out[s] = (raw / den).reshape(B_PC, T, D)
    return out


# revision 28
# speedup vs baseline: 1.2939x; 1.2939x over previous
"""EntityCrossAttention Trainium2 kernel (bf16 streaming, transposed output).

Reference computation (per batch b):
    E = noun_feats[class_ids[b]]            [N, D]
    Q = X @ Wq.T + bq                       [T, D]
    K = E @ Wk.T + bk                       [N, D]
    V = E @ Wv.T + bv                       [N, D]
    S = Q @ K.T / sqrt(D)                   [T, N]
    attn = softmax(S, -1)
    wa = attn * w;  wa /= wa.sum(-1) + 1e-6
    out = wa @ V                            [T, D]

Algebraic restructuring: S = X @ (Wq.T @ K.T)/sqrt(D) + (bq @ K.T)/sqrt(D), so
the [D,D] Q projection never exists on device. Host precomputes per batch:
    M  = Wq.T @ K.T               [D, N]
    eb = (bq @ K.T) / sqrt(D)     [N]
    V' = w[:,None] * V            [N, D]
With unnormalized weights e = exp(S/sqrt(D) + eb):
    out = (e @ V') / (e @ (w + 1e-6))

Device computes the two big contractions only; the tiny per-row denominator
e @ (w+1e-6) is evaluated on the host from the shipped e (bf16, 0.5 MiB/core),
and the final division happens on the host. All PE operands are bf16
(1 col/cycle stream), PSUM accumulation f32, X and outputs stream bf16
(~6e-3 max-rel error vs the f32 reference; tolerance is 2e-2).

Per 1024-row group (8 per core):
    scoresT[n, r] : 2 halves x 4 k-chunk matmuls, M chunks stationary [128,32]
    eT = exp(scoresT*scale + eb) -> bf16 SBUF   (ScalarE, 2 instr)
    raw.T[d, r]   : 4 V' chunks stationary [32,128], eT moving [32,512]
                    -> PSUM [128,512] f32 (6-bank rotation)
    casts PSUM->SBUF bf16 split 5/3 across DVE / ScalarE
The out matmuls for group g issue after the score matmuls of group g+1, so the
PE always has a dense run of 512-column bf16 matmuls (keeps the PE activity
throttle at full rate) and the casts decouple through the 6 PSUM banks.

Sharding: data-parallel over B: 8 cores x 2 batches each. X loads on the SP
HWDGE ring (512 KiB halves, 4 KiB/partition contiguous), raw.T and e stores on
the GpSimd SWDGE ring. Host reassembles and applies the denominator.
"""

import numpy as np

B, T, D, C, N = 16, 4096, 512, 14, 32
N_CORES = 8
B_PC = B // N_CORES          # batches per core
ROWS_PC = B_PC * T           # 8192
GR = 1024                    # rows per group (one 1 MiB DMA each way in bf16)
NG = ROWS_PC // GR           # 8 groups per core
GPB = T // GR                # 4 groups per batch
SH = 512                     # scores half width (PSUM bank / matmul N limit)
KC = D // 128                # 4 contraction chunks
DC = D // 128                # 4 output d-chunks
SCALE = float(D) ** -0.5

_compiled = None


def _build():
    import concourse.bacc as bacc
    import concourse.tile as tile
    import concourse.mybir as mybir

    f32 = mybir.dt.float32
    bf16 = mybir.dt.bfloat16
    Exp = mybir.ActivationFunctionType.Exp
    Copy = mybir.ActivationFunctionType.Copy

    nc = bacc.Bacc("TRN2", debug=False)
    x = nc.dram_tensor("x", [128, NG * 2 * KC * SH], bf16, kind="ExternalInput").ap()
    m = nc.dram_tensor("m", [128, B_PC * KC * N], bf16, kind="ExternalInput").ap()
    vp = nc.dram_tensor("vp", [N, B_PC * D], bf16, kind="ExternalInput").ap()
    eb = nc.dram_tensor("eb", [N, B_PC], f32, kind="ExternalInput").ap()
    out = nc.dram_tensor("out", [128, NG * 2 * DC * SH], bf16,
                         kind="ExternalOutput").ap()
    eo = nc.dram_tensor("eo", [N, NG * GR], bf16, kind="ExternalOutput").ap()

    HCOL = KC * SH  # x columns per half-group load

    with tile.TileContext(nc) as tc:
        with (
            tc.tile_pool(name="const", bufs=1) as cpool,
            tc.tile_pool(name="xin", bufs=2 * NG) as xpool,
            tc.tile_pool(name="et", bufs=NG) as epool,
            tc.tile_pool(name="res", bufs=6) as rpool,
            tc.tile_pool(name="ps_sc", bufs=1, space="PSUM") as ps_sc,
            tc.tile_pool(name="ps_o", bufs=6, space="PSUM") as ps_o,
        ):
            # tiny constants first so the first score matmul is not gated on
            # the full X stream; then queue every X group load on the SP ring
            m_sb = cpool.tile([128, B_PC * KC * N], bf16)
            nc.sync.dma_start(m_sb[:, :], m[:, :])
            vp_sb = cpool.tile([N, B_PC * D], bf16)
            nc.sync.dma_start(vp_sb[:, :], vp[:, :])
            eb_sb = cpool.tile([N, B_PC], f32)
            nc.sync.dma_start(eb_sb[:, :], eb[:, :])
            # ramp: the ACT ring is idle until the first store (~20us), so
            # the first two groups' h1 halves load there in parallel with the
            # SP ring; the PE then never waits for X during warm-up
            x_sb = []
            for gi in range(NG):
                pair = []
                for hh in range(2):
                    xt = xpool.tile([128, HCOL], bf16, name="x_sb", tag="x_sb")
                    j = gi * 2 + hh
                    eng = nc.scalar if (gi < 2 and hh == 1) else nc.sync
                    eng.dma_start(xt[:, :], x[:, j * HCOL : (j + 1) * HCOL])
                    pair.append(xt)
                x_sb.append(pair)

            e_sb = [None] * NG

            # cast split: 5 on DVE, 3 on ScalarE per group of 8
            CAST_ENG = ["v", "v", "s", "v", "s", "v", "s", "v"]

            def scores_stage(gi):
                b = gi // GPB
                e_sb[gi] = epool.tile([N, GR], bf16, name="e_sb", tag="e_sb")
                sc_ps = ps_sc.tile([N, 2 * SH], f32, name="sc_ps", tag="sc_ps")
                for h in range(GR // SH):
                    for k in range(KC):
                        nc.tensor.matmul(
                            sc_ps[:, h * SH : (h + 1) * SH],
                            m_sb[:, (b * KC + k) * N : (b * KC + k + 1) * N],
                            x_sb[gi][h][:, k * SH : (k + 1) * SH],
                            start=(k == 0),
                            stop=(k == KC - 1),
                        )
                # one batched exp over both halves (FD=1024 on ScalarE)
                nc.scalar.activation(
                    e_sb[gi][:, :], sc_ps[:, :], Exp,
                    bias=eb_sb[:, b : b + 1], scale=SCALE,
                )
                # ship e on the SWDGE ring (pure host output)
                nc.gpsimd.dma_start(
                    eo[:, gi * GR : (gi + 1) * GR], e_sb[gi][:, :]
                )

            def out_stage(gi):
                b = gi // GPB
                o_sb = rpool.tile([128, 2 * DC * SH], bf16,
                                  name="o_sb", tag="o_sb")
                last = gi == NG - 1
                for i in range(8):
                    c, ho = divmod(i, 2)
                    o_ps = ps_o.tile([128, SH], f32, name="o_ps", tag="o_ps")
                    nc.tensor.matmul(
                        o_ps[:, :],
                        vp_sb[:, b * D + c * 128 : b * D + (c + 1) * 128],
                        e_sb[gi][:, ho * SH : (ho + 1) * SH],
                        start=True, stop=True,
                    )
                    dst = o_sb[:, i * SH : (i + 1) * SH]
                    # final group: strict alternation drains the tail casts on
                    # both engines in parallel
                    eng_i = ("v" if i % 2 == 0 else "s") if last else CAST_ENG[i]
                    if eng_i == "v":
                        nc.vector.tensor_copy(dst, o_ps[:, :])
                    else:
                        nc.scalar.activation(dst, o_ps[:, :], Copy)
                    # store finished chunks on the SWDGE ring (quarters for
                    # the last group to shorten the drain tail)
                    ci = i + 1
                    step = 2 if gi == NG - 1 else 4
                    if ci % step == 0:
                        hw = ci // step - 1
                        # alternate stores across the SWDGE and SP rings so
                        # the output stream drains at combined rate; SP-ring
                        # stores queue behind the loads and flow once those
                        # finish, keeping ScalarE free for exp+casts
                        deng = nc.gpsimd if hw % 2 == 0 else nc.sync
                        deng.dma_start(
                            out[:, gi * 2 * DC * SH + hw * step * SH
                                : gi * 2 * DC * SH + (hw + 1) * step * SH],
                            o_sb[:, hw * step * SH : (hw + 1) * step * SH],
                        )

            # software pipeline: out matmuls run one group behind scores
            scores_stage(0)
            for gi in range(1, NG):
                scores_stage(gi)
                out_stage(gi - 1)
            out_stage(NG - 1)

    nc.compile()
    return nc


def _get_compiled():
    global _compiled
    if _compiled is None:
        _compiled = _build()
    return _compiled


def kernel(
    visual_feat, noun_feats, class_ids, noun_weights,
    Wq, bq, Wk, bk, Wv, bv,
):
    import ml_dtypes
    from concourse.bass_utils import run_bass_kernel_spmd

    bf = ml_dtypes.bfloat16
    visual_feat = np.asarray(visual_feat, dtype=np.float32)
    noun_feats = np.asarray(noun_feats, dtype=np.float32)
    class_ids = np.asarray(class_ids)
    noun_weights = np.asarray(noun_weights, dtype=np.float32)
    Wq, bq = np.asarray(Wq, np.float32), np.asarray(bq, np.float32)
    Wk, bk = np.asarray(Wk, np.float32), np.asarray(bk, np.float32)
    Wv, bv = np.asarray(Wv, np.float32), np.asarray(bv, np.float32)

    # Host precompute of tiny per-batch constants (all O(B*N*D)).
    E = noun_feats[class_ids]                       # [B, N, D]
    W = noun_weights[class_ids]                     # [B, N]
    Kb = E @ Wk.T + bk                              # [B, N, D]
    Vb = E @ Wv.T + bv                              # [B, N, D]
    M = np.einsum("jd,bnj->bdn", Wq, Kb)            # [B, D, N] = Wq.T @ Kb.T
    ebias = (Kb @ bq) * SCALE                       # [B, N]
    Vp = W[:, :, None] * Vb                         # [B, N, D]
    wpe = W + 1e-6                                  # [B, N]

    nc = _get_compiled()

    in_maps = []
    for c in range(N_CORES):
        s = slice(c * B_PC, (c + 1) * B_PC)
        # m layout: [128, b*KC*N + k*N + n] = M[b, k*128 + p, n]
        m_c = np.ascontiguousarray(
            M[s].reshape(B_PC, KC, 128, N).transpose(2, 0, 1, 3).reshape(128, -1)
        ).astype(bf)
        # x layout: [p, ((gi*2+h)*KC + k)*SH + r] = Xt[k*128+p, gi*GR+h*SH+r]
        xt_c = visual_feat[s].reshape(ROWS_PC, D).T  # [D, ROWS_PC]
        x_c = np.ascontiguousarray(
            xt_c.reshape(KC, 128, NG, 2, SH)
            .transpose(1, 2, 3, 0, 4).reshape(128, -1)
        ).astype(bf)
        in_maps.append(
            {
                "x": x_c,
                "m": m_c,
                "vp": np.ascontiguousarray(
                    Vp[s].transpose(1, 0, 2).reshape(N, B_PC * D)
                ).astype(bf),
                "eb": np.ascontiguousarray(ebias[s].T),
            }
        )

    global _last_in_maps
    _last_in_maps = in_maps
    res = run_bass_kernel_spmd(nc, in_maps, list(range(N_CORES)))
    # defensive: e is exp(.) so every shipped entry must be positive and
    # finite; retry once if a transient device glitch corrupted an output
    def _bad(r):
        for c in range(N_CORES):
            e_c = np.asarray(r.results[c]["eo"]).astype(np.float32)
            o_c = np.asarray(r.results[c]["out"]).astype(np.float32)
            if not (np.isfinite(e_c).all() and (e_c > 0).all()
                    and np.isfinite(o_c).all() and (o_c != 0.0).all()):
                return True
        return False

    if _bad(res):
        res = run_bass_kernel_spmd(nc, in_maps, list(range(N_CORES)))
    out = np.empty((B, T, D), dtype=np.float32)
    for c in range(N_CORES):
        s = slice(c * B_PC, (c + 1) * B_PC)
        # raw.T dram: [p, ((gi*DC + c)*2 + h)*SH + r] = rawT[d=c*128+p,
        # row=gi*GR+h*SH+r]
        o = np.asarray(res.results[c]["out"]).reshape(128, NG, DC, 2, SH)
        raw = (
            o.transpose(1, 3, 4, 2, 0).reshape(ROWS_PC, D).astype(np.float32)
        )
        # e dram: [n, gi*GR + r] -> den[row] = sum_n e[n,row] * wpe[b(row),n]
        e_c = np.asarray(res.results[c]["eo"]).astype(np.float32)  # [N, 8192]
        wpe_c = wpe[s]                                   # [B_PC, N]
        den = np.einsum(
            "nbr,bn->br", e_c.reshape(N, B_PC, T), wpe_c
        ).reshape(ROWS_PC, 1)
        out[s] = (raw / den).reshape(B_PC, T, D)
    return out
